# revision 36
# baseline (speedup 1.0000x reference)
"""Trainium2 Bass kernel for grouped-expert 3-layer MLP (MoE, known covariance).

Computes, for x[B, E, DIN] and per-expert weights:
    h1 = relu(x[:,e] @ W1[e] + b1[e])      # [B, H]
    h2 = relu(h1 @ W2[e] + b2[e])          # [B, H]
    o  = h2 @ W3[e] + b3[e]                # [B, DOUT]
    out = sum_e o                          # [B, DOUT]

Sharding: data-parallel over batch across 8 NeuronCores (B=8192 -> 1024/core).
Weights are replicated to every core; no collectives needed.

Layout strategy (feature-major activations end to end):
  x is pre-transposed per shard on the host to [E, DIN, bloc] and pre-cast to
  bf16 (as are the weights), so every tile loads directly feature-major with
  half the HBM traffic and the PE never spends cycles transposing x.
  All three layers run with the weight panel as the stationary operand and
  activations streaming feature-major:
     h1T[hb]  = relu(W1[:,hb].T @ xT + b1)        4 blocks of [128, NB]
     h2T[gb]  = relu(sum_hb W2[hb,gb].T @ h1T[hb] + b2)
     oT      += sum_gb W3[gb].T @ h2T[gb]         accumulated in PSUM over all
                                                  experts (one bank per batch tile)
  The epilogue adds the e-summed b3 bias, PE-transposes back to batch-major
  and stores contiguously.

Matmul operands are bf16 (activations rounded by the ACT/DVE evacuation op);
PSUM accumulation stays fp32; rel err ~4e-3 vs the fp32 reference.

Performance notes (HW ~186us/8 cores, PE ~89% occupied, stream ~213ns/matmul
= the N=512 1-col/cycle floor):
  - W3 panels are padded/duplicated to 128 stationary columns so every
    matmul runs in the 128x128 array mode; an M=64 output selects the
    128x64 tiling mode and EVERY mode switch drains the PE pipeline
    (~11us lost when L3 interleaves with neighboring 128x128 matmuls).
  - L1+L2 PSUM tiles share one 6-buffer pool (+po's 2 banks = 8 total);
    a fixed 4/2 split loses ~2us to bank-recycle waits.
  - The two batch tiles run phase-shifted through the layers so PSUM
    evacuation latency (ACT/DVE ~0.7us/tile) never gates a matmul.
  - h1 evacuations carry tc.high_priority(): they recycle p1 banks for
    the next expert and would otherwise queue behind h2 evacuations.
  - x tiles ride the sync HWDGE queue (expert 0's split in half), issued
    ahead of the gpsimd weight loads; bias tensors arrive host-prelaid so
    the PE prologue is empty.
"""

import os
from contextlib import ExitStack

import ml_dtypes
import numpy as np

import concourse.tile as tile
from concourse import bacc, mybir
from concourse.bass_utils import run_bass_kernel_spmd
E, DIN, H, DOUT = 16, 128, 512, 64
B_FULL = 8192
N_CORES = 8
HB = H // 128  # 4 h-blocks
F32 = mybir.dt.float32
BF16 = mybir.dt.bfloat16


def build_nc(bloc=B_FULL // N_CORES, nb=512, mm_dt=BF16):
    """Build the per-core Bass program. bloc = local batch, nb = batch tile."""
    nbt = bloc // nb
    nch = nb // 128  # 128-col chunks per batch tile
    hnb = nb // 2  # evacuation half-width
    assert bloc % nb == 0 and nb % 128 == 0

    nc = bacc.Bacc("TRN2", target_bir_lowering=False, debug=False)

    x = nc.dram_tensor("x", [E, DIN, bloc], mm_dt, kind="ExternalInput")
    W1 = nc.dram_tensor("W1", [E, DIN, H], mm_dt, kind="ExternalInput")
    b1 = nc.dram_tensor("b1", [128, HB * E], F32, kind="ExternalInput")
    W2 = nc.dram_tensor("W2", [E, H, H], mm_dt, kind="ExternalInput")
    b2 = nc.dram_tensor("b2", [128, HB * E], F32, kind="ExternalInput")
    W3 = nc.dram_tensor("W3", [E, H, DOUT], mm_dt, kind="ExternalInput")
    b3 = nc.dram_tensor("b3", [DOUT, 1], F32, kind="ExternalInput")
    identd = nc.dram_tensor("ident", [128, 128], F32, kind="ExternalInput")
    out = nc.dram_tensor("out", [bloc, DOUT], F32, kind="ExternalOutput")

    RELU = mybir.ActivationFunctionType.Relu
    ADD = mybir.AluOpType.add
    MAX = mybir.AluOpType.max

    with tile.TileContext(nc) as tc, ExitStack() as ctx:
        consts = ctx.enter_context(tc.tile_pool(name="consts", bufs=1))
        w1p = ctx.enter_context(tc.tile_pool(name="w1p", bufs=2))
        w2p = ctx.enter_context(tc.tile_pool(name="w2p", bufs=2))
        w3p = ctx.enter_context(tc.tile_pool(name="w3p", bufs=2))
        xtp = ctx.enter_context(tc.tile_pool(name="xtp", bufs=4))
        h1p = ctx.enter_context(tc.tile_pool(name="h1p", bufs=3))
        h2p = ctx.enter_context(tc.tile_pool(name="h2p", bufs=3))
        oabp = ctx.enter_context(tc.tile_pool(name="oabp", bufs=2))
        obp = ctx.enter_context(tc.tile_pool(name="obp", bufs=2))
        psp = ctx.enter_context(tc.tile_pool(name="psp", bufs=6, space="PSUM"))
        pop = ctx.enter_context(tc.tile_pool(name="pop", bufs=nbt, space="PSUM"))

        def issue_dmas(e):
            """Queue expert e's x tiles (sync HWDGE) and weights (gpsimd)."""
            xtile = xtp.tile([DIN, nbt, nb], mm_dt, tag="xt", name="xt")
            xs = x[e].rearrange("p (bt n) -> p bt n", bt=nbt)
            if e == 0:
                # split so layer 1's very first matmul gates on half the bytes
                for bt in range(nbt):
                    nc.sync.dma_start(out=xtile[:, bt, :], in_=xs[:, bt, :])
            else:
                nc.sync.dma_start(out=xtile, in_=xs)
            xts = [xtile[:, bt, :] for bt in range(nbt)]
            wdma = nc.gpsimd
            w1t = w1p.tile([DIN, H], mm_dt, tag="w1")
            wdma.dma_start(out=w1t, in_=W1[e])
            w2t = w2p.tile([128, HB, H], mm_dt, tag="w2")
            wdma.dma_start(out=w2t, in_=W2[e].rearrange("(hb p) h -> p hb h", p=128))
            # W3 panels padded to 128 stationary columns: keeps every matmul
            # in 128x128 array mode. An M=64 output would select the 128x64
            # tiling mode, and each mode switch drains the PE pipeline (~8
            # stalls per expert as the scheduler interleaves L3 with
            # neighboring matmuls). The pad half [:, :, 1, :] is left
            # unwritten — its product lands in PSUM partitions 64:127 of
            # po's already-allocated bank and is never read.
            w3t = w3p.tile([128, HB, 2, DOUT], mm_dt, tag="w3")
            wdma.dma_start(
                out=w3t[:, :, 0, :],
                in_=W3[e].rearrange("(hb p) o -> p hb o", p=128),
            )
            return xts, w1t, w2t, w3t

        bufs0 = issue_dmas(0)

        # biases arrive host-prelaid: b1s[p, hb*E+e] = b1[e, hb*128+p] and
        # b3sum pre-summed over experts, so the PE prologue is empty and
        # layer 1's first matmul is the first PE instruction.
        b1s = consts.tile([128, HB * E], F32)
        nc.sync.dma_start(out=b1s, in_=b1[:, :])
        b2s = consts.tile([128, HB * E], F32)
        nc.sync.dma_start(out=b2s, in_=b2[:, :])
        b3sum = consts.tile([DOUT, 1], F32)
        nc.sync.dma_start(out=b3sum, in_=b3[:, :])
        # identity arrives from DRAM (host-built): keeps gpsimd free for
        # weight-DMA descriptor gen at startup; only the epilogue needs it
        ident = consts.tile([128, 128], F32)
        nc.sync.dma_start(out=ident, in_=identd[:, :])

        # PSUM accumulators for the expert-summed output, one per batch tile,
        # alive across the whole expert loop.
        po = [pop.tile([128, nb], F32, tag="po", name=f"po{i}") for i in range(nbt)]

        for e in range(E):
            xts, w1t, w2t, w3t = bufs0 if e == 0 else issue_dmas(e)

            # The two batch tiles run phase-shifted: L1(bt0), L1(bt1),
            # L2(bt0), L2(bt1), L3(bt0), L3(bt1). Each consumer phase starts
            # a full phase (~0.9us) after its producers' evacuations began,
            # so the PE never waits on ACT/DVE latency.
            h1s, h2s = [], []
            for bt in range(nbt):
                # ---- layer 1 ----
                h1 = []
                for hb in range(HB):
                    ps = psp.tile([128, nb], F32, tag="ps")
                    nc.tensor.matmul(
                        ps,
                        w1t[:, hb * 128 : (hb + 1) * 128],
                        xts[bt],
                        start=True,
                        stop=True,
                    )
                    ht = h1p.tile([128, nb], mm_dt, tag=f"h1_{hb}")
                    bias = b1s[:, hb * E + e : hb * E + e + 1]
                    # h1 evacuations gate the p1 bank recycle for the next
                    # expert's L1; prioritize them ahead of queued h2 evacs
                    with tc.high_priority():
                        if hb % 2 == 0:
                            nc.scalar.activation(ht, ps, RELU, bias=bias)
                        else:
                            nc.vector.tensor_scalar(ht, ps, bias, 0.0, ADD, MAX)
                    h1.append(ht)
                h1s.append(h1)

            for bt in range(nbt):
                # ---- layer 2 ----
                h2 = []
                for gb in range(HB):
                    ps = psp.tile([128, nb], F32, tag="ps")
                    for hb in range(HB):
                        nc.tensor.matmul(
                            ps,
                            w2t[:, hb, gb * 128 : (gb + 1) * 128],
                            h1s[bt][hb],
                            start=(hb == 0),
                            stop=(hb == HB - 1),
                        )
                    ht = h2p.tile([128, nb], mm_dt, tag=f"h2_{gb}")
                    bias = b2s[:, gb * E + e : gb * E + e + 1]
                    if gb % 2 == 1:
                        nc.scalar.activation(ht, ps, RELU, bias=bias)
                    else:
                        nc.vector.tensor_scalar(ht, ps, bias, 0.0, ADD, MAX)
                    h2.append(ht)
                h2s.append(h2)

            for bt in range(nbt):
                # ---- layer 3: accumulate over gb and experts in PSUM ----
                for gb in range(HB):
                    nc.tensor.matmul(
                        po[bt],
                        w3t[:, gb, :, :],
                        h2s[bt][gb],
                        start=(e == 0 and gb == 0),
                        stop=(e == E - 1 and gb == HB - 1),
                    )

        # ---- epilogue: bias, transpose back to batch-major, store ----
        for bt in range(nbt):
            b0 = bt * nb
            ob = oabp.tile([DOUT, nb], F32, tag="oab")
            nc.vector.tensor_scalar_add(ob, po[bt][:DOUT, :], b3sum)
            pot = psp.tile([128, nch * DOUT], F32, tag="ps", name="pot")
            for c in range(nch):
                nc.tensor.transpose(
                    pot[:, c * DOUT : (c + 1) * DOUT],
                    ob[:, c * 128 : (c + 1) * 128],
                    ident[:DOUT, :DOUT],
                )
            obt = obp.tile([128, nch * DOUT], F32, tag="obt")
            nc.vector.tensor_copy(obt, pot)
            nc.sync.dma_start(
                out=out[b0 : b0 + nb, :].rearrange("(c p) o -> p c o", p=128),
                in_=obt.rearrange("p (c o) -> p c o", o=DOUT),
            )

    nc.compile()
    return nc


_NC_CACHE = {}


def _get_nc():
    if "nc" not in _NC_CACHE:
        _NC_CACHE["nc"] = build_nc()
    return _NC_CACHE["nc"]


def kernel(x, W1, b1, W2, b2, W3, b3):
    bf16 = ml_dtypes.bfloat16
    x = np.asarray(x, dtype=np.float32)

    def blayout(b):  # [E, H] -> [128, HB*E] with b_out[p, hb*E+e] = b[e, hb*128+p]
        b = np.asarray(b, dtype=np.float32)
        return np.ascontiguousarray(
            b.reshape(E, HB, 128).transpose(2, 1, 0).reshape(128, HB * E)
        )

    ws = {
        "W1": np.ascontiguousarray(np.asarray(W1, dtype=bf16)),
        "b1": blayout(b1),
        "W2": np.ascontiguousarray(np.asarray(W2, dtype=bf16)),
        "b2": blayout(b2),
        "W3": np.ascontiguousarray(np.asarray(W3, dtype=bf16)),
        "b3": np.ascontiguousarray(
            np.asarray(b3, dtype=np.float32).sum(axis=0)[:, None]
        ),
        "ident": np.eye(128, dtype=np.float32),
    }
    nc = _get_nc()
    shards = np.split(x, N_CORES, axis=0)
    # shard to [E, DIN, bloc] (feature-major) so tiles DMA straight to the PE
    in_maps = [
        {"x": np.ascontiguousarray(s.transpose(1, 2, 0).astype(bf16)), **ws}
        for s in shards
    ]
    trace = bool(int(os.environ.get("KERNEL_TRACE", "0")))
    kwargs = {}
    if trace and os.environ.get("KERNEL_TRACE_DIR"):
        kwargs["tmpdir"] = os.environ["KERNEL_TRACE_DIR"]
    res = run_bass_kernel_spmd(nc, in_maps, list(range(N_CORES)), trace=trace, **kwargs)
    if trace:
        kernel.last_results = res
    return np.concatenate([res.results[c]["out"] for c in range(N_CORES)], axis=0)
